# revision 21
# baseline (speedup 1.0000x reference)
"""MatchingNet head (cosine-sim kNN aggregation) on 8 trn2 NeuronCores.

Reference computation:
    sim[m, n] = <fX[m], gS[n]> / max(||fX[m]|| * ||gS[n]||, 1e-8)
    out[m, c] = sum_n sim[m, n] * onehot(trainTarget)[n, c]

Exact algebraic reassociation (the eps guard never binds for D=1024 randn
rows, whose norms concentrate around 32):
    A = gS.T @ (onehot / ||gS||)          # [D, C]
    out = diag(1/||fX||) @ (fX @ A)        # [M, C]

Two SPMD launches (collectives are unavailable under this runtime):
  Phase 1: gS is sharded row-wise; core i computes the partial
           A_i.T = (onehot_i / ||gS_i||).T @ gS_i  over its 512 supports.
           The host sums the eight [64, 1024] fp32 partials and retiles
           the bf16 A for phase 2.
  Phase 2: fX is sharded row-wise (data parallel); each core streams its
           fX.T shard by 128-dim chunks through the PE (A chunk is the
           stationary operand), accumulates OT[c, m] in PSUM, reduces
           sum-of-squares per query with an all-ones [128, 64] stationary
           matmul (which also broadcasts the result to all 64 output
           partitions), applies 1/sqrt via one Abs_reciprocal_sqrt
           activation, and scales OT during the PSUM->SBUF copy. The host
           transposes each core's [64, 1024] slab while gathering.

All matmul operands are bf16 (1 PE cycle/row vs 2-4 for fp32, and half
the HBM traffic); PSUM accumulation stays fp32. Norm squares are fp16
(bf16's 8-bit mantissa would lose ~1% on the chunk-sum accumulation;
fp16's 11 bits keep it ~1e-4, and the sums stay < 2000 << fp16 max).
"""

import numpy as np
from contextlib import ExitStack

import ml_dtypes

import concourse.bass as bass  # noqa: F401
import concourse.tile as tile
import concourse.mybir as mybir
from concourse import bacc, bass2jax
from concourse.bass_utils import run_bass_kernel_spmd

N, D, C, M = 4096, 1024, 64, 8192
NCORES = 8
NS = N // NCORES   # 512 supports per core (phase 1)
MS = M // NCORES   # 1024 queries per core (phase 2)
P = 128
NT = NS // P       # 4 n-tiles per core
DC = D // P        # 8 d-chunks (128 each)
HB = 512           # half width (one PSUM bank of fp32)
F32 = mybir.dt.float32
BF16 = mybir.dt.bfloat16
F16 = mybir.dt.float16
AF = mybir.ActivationFunctionType
MUL = mybir.AluOpType.mult
BF16NP = ml_dtypes.bfloat16

_CACHE = {}


def _build_phase1():
    nc = bacc.Bacc(
        "TRN2", target_bir_lowering=False, debug=False, num_devices=NCORES
    )
    gs = nc.dram_tensor("gs", [P, NT, D], BF16, kind="ExternalInput").ap()
    oh = nc.dram_tensor("oh", [P, NT, C], BF16, kind="ExternalInput").ap()
    atp = nc.dram_tensor("atp", [C, D], BF16, kind="ExternalOutput").ap()

    with tile.TileContext(nc) as tc, ExitStack() as ctx:
        const_pool = ctx.enter_context(tc.tile_pool(name="const", bufs=1))
        sq_pool = ctx.enter_context(tc.tile_pool(name="sqp", bufs=2))
        w_pool = ctx.enter_context(tc.tile_pool(name="wp", bufs=2))
        st_pool = ctx.enter_context(tc.tile_pool(name="stp", bufs=4))
        os_pool = ctx.enter_context(tc.tile_pool(name="osp", bufs=1))
        psA = ctx.enter_context(tc.tile_pool(name="psA", bufs=1, space="PSUM"))

        # DMA issues first in each sequencer's program: per-tile singles so
        # each tile's norm chain starts on first arrival (pair-DMAs delay
        # the completion semaphore to the full 512KB).
        gs_sb = const_pool.tile([P, NT * D], BF16, tag="gs")
        oh_sb = const_pool.tile([P, NT * C], BF16, tag="oh")
        # oh leads: every wt tile needs it, and a late oh was measured to
        # stall the whole matmul stream ~1.5us.
        nc.sync.dma_start(
            oh_sb[:].rearrange("p (t c) -> p t c", t=NT), oh[:, :, :]
        )
        nc.sync.dma_start(gs_sb[:, 0:D], gs[:, 0, :])
        nc.scalar.dma_start(gs_sb[:, D:2 * D], gs[:, 1, :])
        nc.sync.dma_start(gs_sb[:, 2 * D:3 * D], gs[:, 2, :])
        nc.scalar.dma_start(gs_sb[:, 3 * D:4 * D], gs[:, 3, :])
        # Load the abs_rsqrt table now; without this the table load lands in
        # the middle of tile 0's norm chain (Scalar's first ACT would
        # otherwise be a Square, which resolves to a different table).
        dumm = st_pool.tile([1, 1], F32, tag="dumm")
        nc.gpsimd.memset(dumm[:], 1.0)
        dumm2 = st_pool.tile([1, 1], F32, tag="dumm2")
        nc.scalar.activation(dumm2[:], dumm[:], AF.Abs_reciprocal_sqrt)
        # Prime the PE: it only reaches its 2.4GHz pstate after ~3us of
        # continuous work (cold it runs at 1.2GHz). Junk matmuls during the
        # DMA window keep it hot so the real matmuls run ~2x faster.
        junk = const_pool.tile([P, HB], BF16, tag="junk")
        nc.gpsimd.memset(junk[:], 1.0)
        pj = psA.tile([C, HB], F32, tag="junkp", name="pj")
        for i in range(9):
            nc.tensor.matmul(pj[:], junk[:, 0:C], junk[:], start=True,
                             stop=True)

        # Single PSUM tile spanning two banks; each matmul writes one bank.
        # Squares split DVE (STT, tiles 0/2) / Scalar (ACT Square with
        # accum_out, tiles 1/3) so the last tile's chain isn't stuck behind
        # a serial DVE queue. tensor_tensor_reduce is NOT used: it
        # hard-crashes this runtime (NRT_EXEC_UNIT_UNRECOVERABLE).
        pa = psA.tile([C, D], F32, tag="at", name="pa")
        for t in range(NT):
            seg = gs_sb[:, t * D:(t + 1) * D]
            gsq = st_pool.tile([P, 1], F32, tag="gsq")
            sqt = sq_pool.tile([P, D], BF16, tag="sq")
            if t % 2 == 0:
                nc.vector.scalar_tensor_tensor(
                    out=sqt[:], in0=seg, scalar=1.0, in1=seg,
                    op0=MUL, op1=MUL, accum_out=gsq[:],
                )
            else:
                nc.scalar.activation(
                    sqt[:], seg, AF.Square, accum_out=gsq[:]
                )
            grinv = st_pool.tile([P, 1], F32, tag="gr")
            nc.scalar.activation(grinv[:], gsq[:], AF.Abs_reciprocal_sqrt)
            wt = w_pool.tile([P, C], BF16, tag="w")
            nc.vector.tensor_scalar_mul(
                wt[:], oh_sb[:, t * C:(t + 1) * C], grinv[:]
            )
            for h in range(2):
                nc.tensor.matmul(
                    pa[:, h * HB:(h + 1) * HB],
                    wt[:],
                    seg[:, h * HB:(h + 1) * HB],
                    start=(t == 0),
                    stop=(t == NT - 1),
                )
        # Drain PSUM in half-width pieces so the h0 output DMA overlaps the
        # h1 copy (gpsimd can't read PSUM; both copies on DVE).
        o = os_pool.tile([C, D], BF16, tag="o")
        for h in range(2):
            nc.vector.tensor_copy(o[:, h * HB:(h + 1) * HB],
                                  pa[:, h * HB:(h + 1) * HB])
            nc.sync.dma_start(atp[:, h * HB:(h + 1) * HB],
                              o[:, h * HB:(h + 1) * HB])

    nc.compile()
    return nc


def _build_phase2():
    nc = bacc.Bacc(
        "TRN2", target_bir_lowering=False, debug=False, num_devices=NCORES
    )
    a = nc.dram_tensor("a", [P, DC, C], BF16, kind="ExternalInput").ap()
    fxt = nc.dram_tensor("fxt", [P, DC, MS], BF16, kind="ExternalInput").ap()
    out = nc.dram_tensor("out", [C, MS], BF16, kind="ExternalOutput").ap()

    with tile.TileContext(nc) as tc, ExitStack() as ctx:
        const_pool = ctx.enter_context(tc.tile_pool(name="const", bufs=1))
        sq_pool = ctx.enter_context(tc.tile_pool(name="sqp", bufs=3))
        st_pool = ctx.enter_context(tc.tile_pool(name="stp", bufs=2))
        os_pool = ctx.enter_context(tc.tile_pool(name="osp", bufs=1))
        psO = ctx.enter_context(tc.tile_pool(name="psO", bufs=1, space="PSUM"))
        psF = ctx.enter_context(tc.tile_pool(name="psF", bufs=1, space="PSUM"))

        # DMA issues lead both hwdge sequencers' programs. Chunk 0 goes
        # alone (its arrival opens the compute pipeline); the mid-stream
        # bulk goes as pair-DMAs (4KB per-partition lines, ~35% better
        # stream rate); the tail chunks go alone so the norm tail starts
        # on the freshest possible data.
        a_sb = const_pool.tile([P, DC * C], BF16, tag="a")
        fxt_sb = const_pool.tile([P, DC * MS], BF16, tag="fxt")
        nc.scalar.dma_start(
            a_sb[:].rearrange("p (k c) -> p k c", k=DC), a[:, :, :]
        )

        def chunk_ap(k):
            return fxt_sb[:, k * MS:(k + 1) * MS]

        nc.sync.dma_start(chunk_ap(0), fxt[:, 0, :])
        for k0 in (1, 3):
            nc.sync.dma_start(
                fxt_sb[:, k0 * MS:(k0 + 2) * MS].rearrange(
                    "p (t m) -> p t m", t=2),
                fxt[:, k0:k0 + 2, :],
            )
        nc.scalar.dma_start(chunk_ap(5), fxt[:, 5, :])
        nc.scalar.dma_start(chunk_ap(6), fxt[:, 6, :])
        # Chunk 7 lands as column halves: h0's square/add/reduce/scale tail
        # starts while h1 is still in flight.
        nc.sync.dma_start(chunk_ap(7)[:, 0:HB], fxt[:, 7, 0:HB])
        nc.sync.dma_start(chunk_ap(7)[:, HB:MS], fxt[:, 7, HB:MS])

        ones_sb = const_pool.tile([P, C], F16, tag="ones")
        nc.gpsimd.memset(ones_sb[:], 1.0)
        dumm = st_pool.tile([1, 1], F32, tag="dumm")
        nc.gpsimd.memset(dumm[:], 1.0)
        # Dummy arsqrt hoists its table load off the critical tail.
        dumm2 = st_pool.tile([1, 1], F32, tag="dumm2")
        nc.scalar.activation(dumm2[:], dumm[:], AF.Abs_reciprocal_sqrt)
        # (No PE priming here: the pstate decays during the ~1us chunk gaps,
        # and the measured dummies only delayed the first real matmuls.)
        po = psO.tile([C, 2 * HB], F32, tag="ot", name="po")
        pf = psF.tile([C, 2 * HB], F32, tag="fs", name="pf")
        sacc = [
            const_pool.tile([P, MS], F16, tag=f"sacc{j}", name=f"sacc{j}")
            for j in range(2)
        ]

        # Square-engine plan: Scalar (ACT Square) takes half the chunks —
        # it's otherwise idle mid-stream — and DVE owns the accumulators.
        # GpSimd squares are avoided: ~2.4us each AND their SBUF traffic
        # contends with concurrent DVE ops (measured 683 -> 2115ns).
        # Chunk 7's square is split into column halves so the h0 tail
        # chain starts earlier and each tail stage pipelines at half
        # width.
        SCALAR_SQ = (1, 3, 5, 6)
        for k in range(DC):
            chunk = chunk_ap(k)
            # OT[c, m] += A_k[d, c].T @ fX.T[d, m]; A chunk stationary.
            for h in range(2):
                nc.tensor.matmul(
                    po[:, h * HB:(h + 1) * HB],
                    a_sb[:, k * C:(k + 1) * C],
                    chunk[:, h * HB:(h + 1) * HB],
                    start=(k == 0),
                    stop=(k == DC - 1),
                )
            acc = sacc[k // 4]
            if k % 4 == 0:
                dst = acc  # first chunk of each half initializes its acc
            else:
                dst = sq_pool.tile([P, MS], F16, tag="sq", name=f"sq{k}")
            if k in SCALAR_SQ:
                nc.scalar.activation(dst[:], chunk, AF.Square)
            elif k == DC - 1:
                for h in range(2):
                    hs = slice(h * HB, (h + 1) * HB)
                    nc.vector.tensor_mul(dst[:, hs], chunk[:, hs],
                                         chunk[:, hs])
                    nc.vector.tensor_add(acc[:, hs], acc[:, hs], dst[:, hs])
            else:
                nc.vector.tensor_mul(dst[:], chunk, chunk)
            if k % 4 != 0 and k != DC - 1:
                nc.vector.tensor_add(acc[:], acc[:], dst[:])
            # saccA closes after chunk 3: fold it into PSUM mid-stream so
            # only saccB's matmuls sit on the tail.
            if k == 5:
                for h in range(2):
                    nc.tensor.matmul(
                        pf[:, h * HB:(h + 1) * HB],
                        ones_sb[:], sacc[0][:, h * HB:(h + 1) * HB],
                        start=True, stop=False,
                    )

        # Tail, pipelined at half width: pf += ones.T @ saccB, 1/sqrt,
        # scale during the PSUM drain, out-DMA — h0 races ahead of h1.
        frinv = const_pool.tile([C, MS], F32, tag="frinv")
        ot_sb = os_pool.tile([C, MS], BF16, tag="otsb")
        for h in range(2):
            hs = slice(h * HB, (h + 1) * HB)
            nc.tensor.matmul(
                pf[:, hs], ones_sb[:], sacc[1][:, hs],
                start=False, stop=True,
            )
            nc.scalar.activation(frinv[:, hs], pf[:, hs],
                                 AF.Abs_reciprocal_sqrt)
            nc.vector.tensor_mul(ot_sb[:, hs], po[:, hs], frinv[:, hs])
            nc.sync.dma_start(out[:, hs], ot_sb[:, hs])

    nc.compile()
    return nc


def _get_ncs():
    if "nc1" not in _CACHE:
        _CACHE["nc1"] = _build_phase1()
        _CACHE["nc2"] = _build_phase2()
    return _CACHE["nc1"], _CACHE["nc2"]


class _FakeResult:
    def __init__(self, results):
        self.results = results
        self.exec_time_ns = None
        self.instructions_and_trace = None


def _make_runner(nc):
    """One persistently-jitted shard_map executable for this Bass module.

    run_bass_via_pjrt rebuilds its jit closure per call, which retraces and
    re-lowers the HLO every invocation (~3 s/launch of host time). Caching
    the jitted callable keeps warmed kernel() calls fast; the device-side
    NEFF and its execution are identical.
    """
    import jax
    import numpy as _np

    bass2jax.install_neuronx_cc_hook()
    Mesh = bass2jax.Mesh
    PartitionSpec = bass2jax.PartitionSpec
    shard_map = bass2jax.shard_map

    partition_name = (
        nc.partition_id_tensor.name if nc.partition_id_tensor else None
    )
    in_names, out_names, out_avals, zero_shapes = [], [], [], []
    for alloc in nc.m.functions[0].allocations:
        if not isinstance(alloc, mybir.MemoryLocationSet):
            continue
        name = alloc.memorylocations[0].name
        if alloc.kind == "ExternalInput":
            if name != partition_name:
                in_names.append(name)
        elif alloc.kind == "ExternalOutput":
            shape = tuple(alloc.tensor_shape)
            dtype = mybir.dt.np(alloc.dtype)
            out_avals.append(jax.core.ShapedArray(shape, dtype))
            out_names.append(name)
            zero_shapes.append((shape, dtype))
    n_params = len(in_names)
    all_in = list(in_names) + list(out_names)
    if partition_name is not None:
        all_in.append(partition_name)
    donate = tuple(range(n_params, n_params + len(out_names)))

    def _body(*args):
        operands = list(args)
        if partition_name is not None:
            operands.append(bass2jax.partition_id_tensor())
        outs = bass2jax._bass_exec_p.bind(
            *operands,
            out_avals=tuple(out_avals),
            in_names=tuple(all_in),
            out_names=tuple(out_names),
            lowering_input_output_aliases=(),
            sim_require_finite=True,
            sim_require_nnan=True,
            nc=nc,
        )
        return tuple(outs)

    devices = jax.devices()[:NCORES]
    mesh = Mesh(_np.asarray(devices), ("core",))
    nspec = n_params + len(out_names)
    sharded = jax.jit(
        shard_map(
            _body, mesh=mesh,
            in_specs=(PartitionSpec("core"),) * nspec,
            out_specs=(PartitionSpec("core"),) * len(out_names),
            check_rep=False,
        ),
        donate_argnums=donate,
        keep_unused=True,
    )

    def runner(in_maps):
        concat_in = [
            _np.concatenate([_np.asarray(m[name]) for m in in_maps], axis=0)
            for name in in_names
        ]
        concat_zeros = [
            _np.zeros((NCORES * s[0], *s[1:]), dt) for s, dt in zero_shapes
        ]
        out_arrs = sharded(*concat_in, *concat_zeros)
        return _FakeResult([
            {
                name: _np.asarray(out_arrs[i]).reshape(
                    NCORES, *out_avals[i].shape
                )[c]
                for i, name in enumerate(out_names)
            }
            for c in range(NCORES)
        ])

    return runner


def _get_runners():
    if "run1" not in _CACHE:
        nc1, nc2 = _get_ncs()
        _CACHE["run1"] = _make_runner(nc1)
        _CACHE["run2"] = _make_runner(nc2)
    return _CACHE["run1"], _CACHE["run2"]


def _tile_rows(arr, ntiles):
    """[ntiles*128, F] -> [128, ntiles, F] with [p, t, f] = arr[t*128+p, f]."""
    f = arr.shape[1]
    return np.ascontiguousarray(arr.reshape(ntiles, P, f).transpose(1, 0, 2))


def run(gS, fX, trainTarget, nClasses, trace=False, **spmd_kwargs):
    nc1, nc2 = _get_ncs()
    gS = np.asarray(gS, dtype=np.float32).astype(BF16NP)
    fX = np.asarray(fX, dtype=np.float32).astype(BF16NP)
    tt = np.asarray(trainTarget).astype(np.int64).ravel()
    nc_classes = int(np.asarray(nClasses))
    assert nc_classes == C and gS.shape == (N, D) and fX.shape == (M, D)

    oh = np.zeros((N, C), dtype=BF16NP)
    oh[np.arange(N), tt] = 1.0

    in_maps1 = []
    for i in range(NCORES):
        gsl = gS[i * NS:(i + 1) * NS]
        osl = oh[i * NS:(i + 1) * NS]
        in_maps1.append(
            {"gs": _tile_rows(gsl, NT), "oh": _tile_rows(osl, NT)}
        )
    if trace or spmd_kwargs:
        res1 = run_bass_kernel_spmd(
            nc1, in_maps1, core_ids=list(range(NCORES)), trace=trace,
            **spmd_kwargs
        )
    else:
        res1 = _get_runners()[0](in_maps1)
    # gather-reduce the partial A.T's, retile to [128, 8, 64] bf16
    at = np.zeros((C, D), dtype=np.float32)
    for i in range(NCORES):
        at += res1.results[i]["atp"].astype(np.float32)
    a_tiled = _tile_rows(np.ascontiguousarray(at.T.astype(BF16NP)), DC)

    in_maps2 = []
    for i in range(NCORES):
        sl = fX[i * MS:(i + 1) * MS]                       # [MS, D] bf16
        fxt_tiled = np.ascontiguousarray(
            sl.T.reshape(DC, P, MS).transpose(1, 0, 2)
        )
        in_maps2.append({"a": a_tiled, "fxt": fxt_tiled})
    if trace or spmd_kwargs:
        res2 = run_bass_kernel_spmd(
            nc2, in_maps2, core_ids=list(range(NCORES)), trace=trace,
            **spmd_kwargs
        )
    else:
        res2 = _get_runners()[1](in_maps2)
    outs = [
        np.ascontiguousarray(res2.results[i]["out"].T).astype(np.float32)
        for i in range(NCORES)
    ]
    full = np.concatenate(outs, axis=0)
    return full, (res1, res2)


def kernel(gS, fX, trainTarget, nClasses):
    full, _ = run(gS, fX, trainTarget, nClasses)
    return full
